# revision 77
# baseline (speedup 1.0000x reference)
"""Trainium2 Bass kernel for a 2-layer GraphConv GCN (nn_GCNN_69776038691375).

reference semantics:
    x = h.swapaxes(0,1)                       # [N, B, F]
    out_deg/in_deg from src/dst, clipped at 1
    s = out_deg**-0.5 ; d = in_deg**-0.5
    layer(x, W, b) = (segsum((x*s)[src] -> dst) * d) @ W + b
    y = relu(layer(x, W1, b1)); out = layer(y, W2, b2); return out.swapaxes(0,1)

Key identities: aggregation commutes with the feature transform and the
per-node scales fold into the tables, so
    table1 = (x @ W1) * s            (bf16, built shard-local, AllGathered)
    y1     = relu(agg1(table1) * d + b1)
    table2 = (y1 @ W2) * s           (bf16, AllGathered)
    out    = agg2(table2) * d + b2

Distribution (8 cores): destination-node sharding. Nodes padded to
NPAD=50176 = 8 cores x 49 blocks x 128. Core c owns blocks [c*49,(c+1)*49).
Each core transforms only its own node shard (phase A). Tables are exchanged
in 4 block-range chunks, each AllGathered as soon as its blocks are built so
aggregation gathers overlap the producing phase (chunking also keeps
dma_gather int16 indices in range). Aggregations gather per-edge table rows
(bf16, 512B/256B descriptors) and reduce with one-hot matrices built on
device (is_equal vs an iota), accumulating in PSUM via bf16 TensorE matmuls
(1 cycle/row vs 4 for fp32). The aggregation output is kept feature-major
[(b,f), node] so the W2 transform is a direct matmul (no PE transposes; W2
is laid out block-diagonal since PE rejects operands based at partition 64);
d-norm is applied per-column from a host-replicated tile; degree norms come
precomputed from the host (graph-structure preprocessing, same class as the
edge sorting/index tables)."""

import ml_dtypes
import numpy as np

import concourse.bacc as bacc
import concourse.bass as bass
import concourse.mybir as mybir
import concourse.tile as tile
from concourse.bass_interp import get_hw_module
from concourse.bass_utils import run_bass_kernel_spmd

F32 = mybir.dt.float32
BF16 = mybir.dt.bfloat16
I16 = mybir.dt.int16
NPBF = ml_dtypes.bfloat16

# problem sizes (hardcoded per contract)
N = 50000
E = 800000
B = 4
IN_D, HID_D, OUT_D = 64, 64, 32
NCORES = 8
PB = 49                  # blocks per core
NB = NCORES * PB         # 392 global blocks
NPAD = NB * 128          # 50176
CHUNK = PB * 128         # 6272 nodes per core
D1 = B * HID_D           # 256 floats per layer-1 table row
D2 = B * OUT_D           # 128 floats per layer-2 table row
SENT = 250               # one-hot sentinel for padded edge slots
G = 3                    # blocks per gather/compute group
CB = [0, 8, 24, 44, 49]                  # table chunk boundaries (block index)
NCK = len(CB) - 1                    # 4 chunks
NBLK = [CB[r + 1] - CB[r] for r in range(NCK)]
LROWS = [nb * 128 for nb in NBLK]    # local rows per chunk
NH1 = (NCK + 1) // 2                 # chunks [0, NH1) share one-hot tile A


def _groups():
    return [list(range(i, min(i + G, PB))) for i in range(0, PB, G)]


# ---------------------------------------------------------------- host side

def _wrap_idx(flat):
    """dma_gather index layout: idx j of a gather lives at [j%16, j//16],
    replicated across the 8 groups of 16 partitions. flat: [T, 128] int16
    (subtile-major). Returns [128, T*8]."""
    T = flat.shape[0]
    w = flat.reshape(T, 8, 16).transpose(2, 0, 1).reshape(16, T * 8)
    return np.tile(w, (8, 1)).astype(np.int16)


def _preprocess(src, dst):
    """Edge structure + degree norms. One ordering shared by both layers:
    edges sorted by (dst block, src table chunk, src); subtile stream is
    grouped [chunk0(b0..b3) | chunk1(b0..b3) | ...] per G-block group."""
    src = np.asarray(src).astype(np.int64)
    dst = np.asarray(dst).astype(np.int64)

    out_deg = np.bincount(src, minlength=NPAD).astype(np.float32)
    in_deg = np.bincount(dst, minlength=NPAD).astype(np.float32)
    s = 1.0 / np.sqrt(np.maximum(out_deg, 1.0))
    d = 1.0 / np.sqrt(np.maximum(in_deg, 1.0))

    src_c = src // CHUNK
    src_b = (src % CHUNK) >> 7
    src_p = src & 127
    ck = np.searchsorted(CB, src_b, side='right') - 1     # chunk of src
    lo = np.asarray(CB)[ck]
    lrows = np.asarray(LROWS)[ck]
    nblk = np.asarray(NBLK)[ck]
    # p-major row order within a chunk: row = p*nblk + (b-lo). Consecutive
    # blocks of one partition are then contiguous in DRAM, so the staged
    # table writes use >=1KB descriptors (256B rows alone pay a 2x penalty)
    pos = src_c * lrows + src_p * nblk + (src_b - lo)     # row in full chunk
    blk = dst >> 7
    order = np.lexsort((src, ck, blk))
    t_pos, t_dst, t_blk, t_ck = pos[order], dst[order], blk[order], ck[order]
    cnt = np.bincount(t_blk * NCK + t_ck, minlength=NB * NCK).reshape(NB, NCK)
    starts = np.concatenate([[0], np.cumsum(cnt.ravel())])[:-1].reshape(NB, NCK)
    # subtile counts per (chunk, block index), max over cores (shared shape)
    Cc = [(-(-cnt[:, r] // 128)).reshape(NCORES, PB).max(axis=0).astype(int)
          for r in range(NCK)]

    groups = _groups()
    T = int(sum(int(c.sum()) for c in Cc))
    CMAXG = max(max(int(sum(Cc[r][g].sum() for r in range(NH1))),
                    int(sum(Cc[r][g].sum() for r in range(NH1, NCK))))
                for g in [np.array(grp) for grp in groups])

    percore = []
    for c in range(NCORES):
        gsl, dsl = [], []
        for grp in groups:
            for r in range(NCK):
                for b in grp:
                    g = c * PB + b
                    n = int(cnt[g, r])
                    st = int(starts[g, r])
                    C = int(Cc[r][b])
                    gi = np.zeros(C * 128, np.int16)
                    dl = np.full(C * 128, SENT, np.int16)
                    gi[:n] = t_pos[st:st + n].astype(np.int16)
                    dl[:n] = (t_dst[st:st + n] - g * 128).astype(np.int16)
                    gsl.append(gi.reshape(C, 128))
                    dsl.append(dl.reshape(C, 128))
        gs = np.concatenate(gsl, axis=0)
        ds = np.concatenate(dsl, axis=0)
        sc = s[c * CHUNK:(c + 1) * CHUNK]
        dc = d[c * CHUNK:(c + 1) * CHUNK]
        percore.append({
            "gidx": _wrap_idx(gs),                              # [128, T*8]
            "dstl": np.ascontiguousarray(ds.T).astype(NPBF),    # [128, T]
            "sloc": np.ascontiguousarray(sc.reshape(PB, 128).T),  # [128, PB]
            "drep": np.tile(dc, (128, 1)).astype(NPBF),         # [128, CHUNK]
        })
    meta = dict(Cc=tuple(tuple(int(x) for x in c) for c in Cc),
                T=T, CMAXG=CMAXG)
    return percore, meta


# -------------------------------------------------------------- bass program

def _build(meta, collectives=True, upto='l2'):
    Cc = meta["Cc"]
    T, CMAXG = meta["T"], meta["CMAXG"]
    groups = _groups()
    toff = np.concatenate(
        [[0], np.cumsum([sum(Cc[r][b] for r in range(NCK) for b in grp)
                         for grp in groups])]).astype(int)

    nc = bacc.Bacc("TRN2", target_bir_lowering=False, debug=False,
                   num_devices=NCORES)

    hTl = nc.dram_tensor("hTl", [B, IN_D, CHUNK], BF16, kind="ExternalInput")
    w1 = nc.dram_tensor("w1", [IN_D, HID_D], BF16, kind="ExternalInput")
    # block-diagonal [[W2, 0], [0, W2]]: one K=128 matmul transforms a
    # 2-batch feature-major y1 tile (PE rejects operands based at part. 64)
    w2 = nc.dram_tensor("w2", [128, 2 * OUT_D], BF16, kind="ExternalInput")
    b1r = nc.dram_tensor("b1r", [128, 1], F32, kind="ExternalInput")
    b2r = nc.dram_tensor("b2r", [128, 1], F32, kind="ExternalInput")
    sloc = nc.dram_tensor("sloc", [128, PB], F32, kind="ExternalInput")
    drep = nc.dram_tensor("drep", [128, CHUNK], BF16, kind="ExternalInput")
    gidx = nc.dram_tensor("gidx", [128, T * 8], I16, kind="ExternalInput")
    dstl = nc.dram_tensor("dstl", [128, T], BF16, kind="ExternalInput")

    out_loc = nc.dram_tensor("out_loc", [128, CHUNK], BF16,
                             kind="ExternalOutput")

    xw1_loc = [nc.dram_tensor(f"xw1_loc_{r}", [LROWS[r], D1], BF16)
               for r in range(NCK)]
    xw1_full = [nc.dram_tensor(f"xw1_full_{r}", [NCORES * LROWS[r], D1], BF16,
                               addr_space="Shared") for r in range(NCK)]
    y2w_loc = [nc.dram_tensor(f"y2w_loc_{r}", [LROWS[r], D2], BF16)
               for r in range(NCK)]
    y2w_full = [nc.dram_tensor(f"y2w_full_{r}", [NCORES * LROWS[r], D2], BF16,
                               addr_space="Shared") for r in range(NCK)]

    rg = [list(range(NCORES))]

    def exchange(loc, full, rows, eng=None):
        if collectives:
            nc.gpsimd.collective_compute(
                "AllGather", mybir.AluOpType.bypass, replica_groups=rg,
                ins=[loc[:]], outs=[full[:]])
        else:
            e = eng or nc.sync
            for c in range(NCORES):
                e.dma_start(out=full[c * rows:(c + 1) * rows, :],
                            in_=loc[:])

    def ck_of(b):
        return next(r for r in range(NCK) if CB[r] <= b < CB[r + 1])

    with tile.TileContext(nc) as tc:
        with (
            tc.tile_pool(name="persist", bufs=1) as pp,
            tc.tile_pool(name="sbuf", bufs=2) as sb,
            tc.tile_pool(name="post", bufs=3) as pq,
            tc.tile_pool(name="psA", bufs=2, space="PSUM") as psA,
            tc.tile_pool(name="psB", bufs=2, space="PSUM") as psB,
            tc.tile_pool(name="psB2", bufs=2, space="PSUM") as psB2,
            tc.tile_pool(name="psC", bufs=2, space="PSUM") as psC,
        ):
            # ---- constants / persistent state
            gidx_t = pp.tile([128, T * 8], I16)
            nc.scalar.dma_start(out=gidx_t[:], in_=gidx[:])
            dstl_t = pp.tile([128, T], BF16)
            nc.scalar.dma_start(out=dstl_t[:], in_=dstl[:])
            w1_t = pp.tile([IN_D, HID_D], BF16)
            nc.sync.dma_start(out=w1_t[:], in_=w1[:])
            w2_t = pp.tile([128, 2 * OUT_D], BF16)
            nc.sync.dma_start(out=w2_t[:], in_=w2[:])
            b1_t = pp.tile([128, 1], F32)
            nc.sync.dma_start(out=b1_t[:], in_=b1r[:])
            b2_t = pp.tile([128, 1], F32)
            nc.sync.dma_start(out=b2_t[:], in_=b2r[:])
            s_t = pp.tile([128, PB], F32)
            nc.sync.dma_start(out=s_t[:], in_=sloc[:])
            d_rep = pp.tile([128, CHUNK], BF16)
            nc.sync.dma_start(out=d_rep[:], in_=drep[:])
            # jr[p, c, t] = c  (exact in bf16 for c < 128); subtile-last
            # layout keeps every one-hot operand's last dim packed 2-byte,
            # which enables the DVE 2x mode (a stride-0 LAST dim would not)
            jr_t = pp.tile([128, 128, CMAXG], BF16)
            nc.gpsimd.iota(jr_t[:], pattern=[[1, 128], [0, CMAXG]],
                           channel_multiplier=0,
                           allow_small_or_imprecise_dtypes=True)

            # ---- phase A: local transform  table1 = (x @ W1) * s  (bf16)
            GA = 4
            for g0 in range(0, PB, GA):
                L = min(GA, PB - g0)
                lhs = sb.tile([IN_D, B * GA * 128], BF16, tag="pa_lhs")
                for bb in range(B):
                    nc.sync.dma_start(
                        out=lhs[:, bb * GA * 128:bb * GA * 128 + L * 128],
                        in_=hTl[bb, :, g0 * 128:(g0 + L) * 128])
                st = sb.tile([128, GA * D1], BF16, tag="pa_st")
                for k in range(L):
                    b = g0 + k
                    ps = psA.tile([128, D1], F32, space="PSUM", tag="paps")
                    for bb in range(B):
                        nc.tensor.matmul(
                            ps[:, bb * HID_D:(bb + 1) * HID_D],
                            lhsT=lhs[:, bb * GA * 128 + k * 128:
                                     bb * GA * 128 + (k + 1) * 128],
                            rhs=w1_t[:], start=True, stop=True)
                    nc.vector.tensor_scalar_mul(
                        st[:, k * D1:(k + 1) * D1], ps[:], s_t[:, b:b + 1])
                r = ck_of(g0)
                nc.sync.dma_start(
                    out=xw1_loc[r][:, :].rearrange(
                        "(p c) f -> p c f", c=NBLK[r])[
                        :, g0 - CB[r]:g0 - CB[r] + L, :],
                    in_=st[:, :L * D1])
                if g0 + L == CB[r + 1]:
                    exchange(xw1_loc[r], xw1_full[r], LROWS[r], eng=nc.scalar)

            # ---- shared per-group aggregation machinery
            def agg_group(gi, grp, tabs, D, onehot=True, gtag="gT"):
                """Chunked gathers + two one-hot builds for group gi.
                Subtile stream: [ck0(blocks) | ck1 | ck2 | ck3]."""
                base = int(toff[gi])
                sC = [sum(Cc[r][b] for b in grp) for r in range(NCK)]
                sCt = sum(sC)
                sH1 = sum(sC[:NH1])
                gT = sb.tile([128, max(sCt, 1), D], BF16, tag=gtag)
                o = 0
                for r in range(NCK):
                    if sC[r]:
                        nc.gpsimd.dma_gather(
                            out_ap=gT[:, o:o + sC[r], :], in_ap=tabs[r][:],
                            idxs_ap=gidx_t[:, (base + o) * 8:
                                           (base + o + sC[r]) * 8],
                            num_idxs=sC[r] * 128, num_idxs_reg=sC[r] * 128,
                            elem_size=D, single_packet=False)
                    o += sC[r]
                ohA = sb.tile([128, 128, max(sH1, 1)], BF16, tag="oh", bufs=4)
                if sH1 and onehot:
                    nc.vector.tensor_tensor(
                        out=ohA[:, :, :sH1],
                        in0=dstl_t[:, base:base + sH1].unsqueeze(1)
                            .broadcast_to([128, 128, sH1]),
                        in1=jr_t[:, :, :sH1], op=mybir.AluOpType.is_equal)
                ohB = sb.tile([128, 128, max(sCt - sH1, 1)], BF16, tag="oh",
                              bufs=4)
                if sCt - sH1 and onehot:
                    nc.vector.tensor_tensor(
                        out=ohB[:, :, :sCt - sH1],
                        in0=dstl_t[:, base + sH1:base + sCt].unsqueeze(1)
                            .broadcast_to([128, 128, sCt - sH1]),
                        in1=jr_t[:, :, :sCt - sH1], op=mybir.AluOpType.is_equal)
                return gT, ohA, ohB, sC, sH1

            def block_subtiles(grp, k, sC, sH1, ohA, ohB):
                """(gathered column, one-hot tile, one-hot column) triples
                for block grp[k] of the group."""
                b = grp[k]
                seq = []
                o = 0
                for r in range(NCK):
                    boff = o + sum(Cc[r][grp[j]] for j in range(k))
                    for j in range(Cc[r][b]):
                        gc = boff + j
                        if r < NH1:
                            seq.append((gc, ohA, gc))
                        else:
                            seq.append((gc, ohB, gc - sH1))
                    o += sC[r]
                return seq

            # ---- phase B: L1 aggregation + table2 build
            LV = {'pa': 0, 'g1': 1, 'o1': 2, 'm1': 3, 'p1': 4,
                  't1': 4.5, 'l1': 5, 'l2': 6}.get(upto, 0)
            for gi, grp in enumerate(groups if LV >= 1 else []):
                g0, L = grp[0], len(grp)
                gT, ohA, ohB, sC, sH1 = agg_group(gi, grp, xw1_full, D1,
                                                  onehot=(LV >= 2))
                y2st = sb.tile([128, G * D2], BF16, tag="y2st", bufs=3)
                for k, b in enumerate(grp):
                    if LV < 3:
                        continue
                    seq = block_subtiles(grp, k, sC, sH1, ohA, ohB)
                    agg0t = psB.tile([128, 128], F32, space="PSUM",
                                     tag="agg0", name="agg0t")
                    agg1t = psB2.tile([128, 128], F32, space="PSUM",
                                      tag="agg1", name="agg1t")
                    agg0, agg1 = agg0t[:], agg1t[:]
                    for i, (gc, oht, oc) in enumerate(seq):
                        fl = dict(start=(i == 0), stop=(i == len(seq) - 1))
                        nc.tensor.matmul(agg0, lhsT=gT[:, gc, 0:128],
                                         rhs=oht[:, :, oc], **fl)
                        nc.tensor.matmul(agg1, lhsT=gT[:, gc, 128:256],
                                         rhs=oht[:, :, oc], **fl)
                    if LV < 4:
                        continue
                    dsl = d_rep[:, b * 128:(b + 1) * 128]
                    y10 = pq.tile([128, 128], F32, tag="y10")
                    nc.vector.tensor_tensor(out=y10[:], in0=agg0, in1=dsl,
                                            op=mybir.AluOpType.mult)
                    y11 = pq.tile([128, 128], F32, tag="y11")
                    nc.vector.tensor_tensor(out=y11[:], in0=agg1, in1=dsl,
                                            op=mybir.AluOpType.mult)
                    y10r = pq.tile([128, 128], BF16, tag="y10r")
                    nc.scalar.activation(y10r[:], y10[:],
                                         mybir.ActivationFunctionType.Relu,
                                         bias=b1_t[:])
                    y11r = pq.tile([128, 128], BF16, tag="y11r")
                    nc.scalar.activation(y11r[:], y11[:],
                                         mybir.ActivationFunctionType.Relu,
                                         bias=b1_t[:])
                    if LV < 4.5:
                        continue
                    tf = psC.tile([128, D2], F32, space="PSUM", tag="tf")
                    nc.tensor.matmul(tf[:, 0:2 * OUT_D], lhsT=y10r[:],
                                     rhs=w2_t[:], start=True, stop=True)
                    nc.tensor.matmul(tf[:, 2 * OUT_D:D2], lhsT=y11r[:],
                                     rhs=w2_t[:], start=True, stop=True)
                    nc.vector.tensor_scalar_mul(
                        y2st[:, k * D2:(k + 1) * D2], tf[:], s_t[:, b:b + 1])
                if LV < 5:
                    continue
                r = ck_of(g0)
                nc.scalar.dma_start(
                    out=y2w_loc[r][:, :].rearrange(
                        "(p c) f -> p c f", c=NBLK[r])[
                        :, g0 - CB[r]:g0 - CB[r] + L, :],
                    in_=y2st[:, :L * D2])
                if g0 + L == CB[r + 1]:
                    exchange(y2w_loc[r], y2w_full[r], LROWS[r])

            # ---- phase C: L2 aggregation -> output
            for gi, grp in enumerate(groups if LV >= 6 else []):
                g0, L = grp[0], len(grp)
                gT, ohA, ohB, sC, sH1 = agg_group(gi, grp, y2w_full, D2)
                ost = sb.tile([128, G * 128], BF16, tag="ost", bufs=3)
                for k, b in enumerate(grp):
                    seq = block_subtiles(grp, k, sC, sH1, ohA, ohB)
                    agg0t = psB.tile([128, 128], F32, space="PSUM",
                                     tag="agg0", name="agg0t")
                    agg0 = agg0t[:]
                    for i, (gc, oht, oc) in enumerate(seq):
                        nc.tensor.matmul(agg0, lhsT=gT[:, gc, 0:128],
                                         rhs=oht[:, :, oc], start=(i == 0),
                                         stop=(i == len(seq) - 1))
                    oa = pq.tile([128, 128], F32, tag="oa")
                    nc.vector.tensor_tensor(
                        out=oa[:], in0=agg0,
                        in1=d_rep[:, b * 128:(b + 1) * 128],
                        op=mybir.AluOpType.mult)
                    nc.vector.tensor_scalar_add(
                        ost[:, k * 128:(k + 1) * 128], oa[:], b2_t[:])
                nc.scalar.dma_start(
                    out=out_loc[:, g0 * 128:(g0 + L) * 128],
                    in_=ost[:, :L * 128])

    nc.compile()
    return nc


# ------------------------------------------------------------------- driver

def _prepare_inputs(h, W1, b1, W2, b2, src, dst):
    percore, meta = _preprocess(src, dst)
    hP = np.zeros((B, NPAD, IN_D), np.float32)
    hP[:, :N, :] = np.asarray(h, np.float32)
    b1r = np.tile(np.asarray(b1, np.float32), 2).reshape(128, 1)
    b2r = np.tile(np.asarray(b2, np.float32), 4).reshape(128, 1)
    common = {
        "w1": np.asarray(W1, np.float32).astype(NPBF),
        "w2": np.kron(np.eye(2, dtype=np.float32),
                      np.asarray(W2, np.float32)).astype(NPBF),
        "b1r": b1r, "b2r": b2r,
    }
    in_maps = []
    for c in range(NCORES):
        hTl = np.ascontiguousarray(
            hP[:, c * CHUNK:(c + 1) * CHUNK, :].transpose(0, 2, 1)
        ).astype(NPBF)
        in_maps.append(dict(common, hTl=hTl, **percore[c]))
    return in_maps, meta


_BUILD_CACHE = {}


def _get_nc(meta):
    key = tuple(sorted((k, tuple(v) if isinstance(v, list) else v)
                       for k, v in meta.items()))
    if key not in _BUILD_CACHE:
        nc = _build(meta)
        nc.m = get_hw_module(nc.m)
        _BUILD_CACHE[key] = nc
    return _BUILD_CACHE[key]


def _assemble(results):
    full = np.concatenate(
        [results[c]["out_loc"].astype(np.float32) for c in range(NCORES)],
        axis=1)                                         # [128, NPAD]
    out = full.reshape(B, OUT_D, NPAD)[:, :, :N].transpose(0, 2, 1)
    return np.ascontiguousarray(out, dtype=np.float32)


def kernel(h, W1, b1, W2, b2, src, dst):
    in_maps, meta = _prepare_inputs(h, W1, b1, W2, b2, src, dst)
    nc = _get_nc(meta)
    res = run_bass_kernel_spmd(nc, in_maps, core_ids=list(range(NCORES)))
    return _assemble(res.results)
